# revision 38
# baseline (speedup 1.0000x reference)
"""MIL gated-attention pooling kernel for Trainium2 (8 NeuronCores, SPMD).

Problem (per reference):
    A_pre = tanh(x@W1 + b1) * sigmoid(x@W3 + b3)      # [N, H]
    A     = A_pre @ W2 + b2                           # [N, K]
    P     = softmax over instances per (bag, head)    # [B, K, L]
    out   = einsum('bkl,bld->bkd', P, x) -> [B, K*D]

Shapes hardcoded: B=32 bags, L=2048 instances/bag, D=512, H=256, K=4.
Sharding: data-parallel over bags, 4 bags (8192 rows) per core, weights
replicated. No cross-core communication.

Device design (v2):
  - All input DMAs are issued upfront from the sync queue in consumption
    order; HBM tensors are pre-tiled on the host so each transfer is a
    few KB contiguous per partition (the DGE fans one dma_start across
    all 16 hardware queues).
  - Logits path: fp8 DoubleRow matmuls against a host-pretransposed,
    pre-packed xT; sigmoid(h) = 0.5*(1+tanh(h/2)) with the 0.5 folded
    into W2; b2 and softmax max-subtraction dropped (cancel in softmax).
  - NTILE=1024 instances per pipeline step; each activation covers a
    [128, 2, 512] PSUM span (two banks) to amortize the ~350ns fixed
    ACT cost.
  - Softmax denominator comes from one ones-vector matmul per bag over
    the exp tile; normalization (u/z) happens on the host, so the
    device tail is just two small copies + DMAs per bag.
  - Two-level software pipeline: the A=ap@W2 matmuls of n-tile i run
    during iteration i+1, the exp-weighted sums of n-tile i during
    iteration i+2, interleaved so PE weight loads hide under streams.
  - FP8_WSUM: weighted sum also in fp8 DoubleRow (x and exp(A) both
    fp8) — halves both its PE streaming time and the xa HBM traffic.
"""

import numpy as np
import ml_dtypes
from contextlib import ExitStack

B, L, D, H, K = 32, 2048, 512, 256, 4
NCORES = 8
BPC = B // NCORES        # bags per core = 4
R = BPC * L              # rows per core = 8192
NTILE = 1024             # instances per pipeline step
NT = R // NTILE          # n-tiles per core = 8
NTB = 2                  # n-tiles per bag
CPT = NTILE // 128       # 128-row chunks per n-tile = 8

_BF16 = ml_dtypes.bfloat16
_FP8 = ml_dtypes.float8_e4m3
FP8_WSUM = True          # fp8 DoubleRow for the weighted sum too
_CACHE = {}


def _build_nc():
    import concourse.bacc as bacc
    import concourse.tile as tile
    import concourse.mybir as mybir
    import concourse.bass as bass

    dt = mybir.dt
    AF = mybir.ActivationFunctionType
    DR = mybir.MatmulPerfMode.DoubleRow

    nc = bacc.Bacc("TRN2", target_bir_lowering=False, debug=False)
    # [nt, p, 2*dc2+r, n] = xT[dc2*256 + r*128 + p, nt*NTILE + n] (fp8, x*1)
    xt = nc.dram_tensor("xt", [NT, 128, 4, NTILE], dt.float8e4, kind="ExternalInput").ap()
    if FP8_WSUM:
        # [g, p, jp, r, d] = x[g*NTILE + jp*256 + r*128 + p, d] (fp8)
        xa = nc.dram_tensor("xa", [NT, 128, CPT // 2, 2, D], dt.float8e4, kind="ExternalInput").ap()
    else:
        # [g, p, j, d] = x[g*NTILE + j*128 + p, d] (bf16)
        xa = nc.dram_tensor("xa", [NT, 128, CPT, D], dt.bfloat16, kind="ExternalInput").ap()
    # [p, dc2, r, h'] = 16*W13[dc2*256 + r*128 + p, h']
    w13 = nc.dram_tensor("w13", [128, 2, 2, 2 * H], dt.float8e4, kind="ExternalInput").ap()
    w2 = nc.dram_tensor("w2", [128, 2, K], dt.bfloat16, kind="ExternalInput").ap()
    b13 = nc.dram_tensor("b13", [128, 4], dt.float32, kind="ExternalInput").ap()
    uout = nc.dram_tensor("uout", [K, BPC, D], dt.float32, kind="ExternalOutput").ap()
    zshape = [128, BPC, 2] if FP8_WSUM else [16 * K, BPC]
    zout = nc.dram_tensor("zout", zshape, dt.float32, kind="ExternalOutput").ap()

    with tile.TileContext(nc) as tc, ExitStack() as ctx:
        consts = ctx.enter_context(tc.tile_pool(name="consts", bufs=1))
        tsp = ctx.enter_context(tc.tile_pool(name="tsp", bufs=4))
        app = ctx.enter_context(tc.tile_pool(name="app", bufs=4))
        epool = ctx.enter_context(tc.tile_pool(name="epool", bufs=2))
        opool = ctx.enter_context(tc.tile_pool(name="opool", bufs=2))

        psH = ctx.enter_context(tc.tile_pool(name="psH", bufs=3, space=bass.MemorySpace.PSUM))
        psA = ctx.enter_context(tc.tile_pool(name="psA", bufs=1, space=bass.MemorySpace.PSUM))
        psU = ctx.enter_context(tc.tile_pool(name="psU", bufs=1, space=bass.MemorySpace.PSUM))

        # SBUF-resident inputs: everything fits, so issue every input DMA
        # upfront in consumption order (each one fans out over the DGE
        # queues; completion order tracks trigger order).
        w13_sb = consts.tile([128, 2, 2, 2 * H], dt.float8e4)
        w2_sb = consts.tile([128, 2, K], dt.bfloat16)
        b13_sb = consts.tile([128, 4], dt.float32)
        xt_sb = consts.tile([128, NT, 4, NTILE], dt.float8e4)
        if FP8_WSUM:
            xa_sb = consts.tile([128, NT, CPT // 2, 2, D], dt.float8e4)
        else:
            xa_sb = consts.tile([128, NT, CPT, D], dt.bfloat16)
        ones_sb = consts.tile([128, 1], dt.float8e4 if FP8_WSUM else dt.bfloat16)

        # the dummy tanh is scalar's first instruction: it pulls the
        # ~1.3us activation-table load into the input-DMA wait window
        # instead of letting it block the first real activation.
        scratch = consts.tile([128, 1], dt.float32)
        nc.scalar.activation(scratch[:], scratch[:], AF.Tanh)
        # PE warmup: dummy matmuls on zeroed scratch fill the first-DMA
        # wait window and start the HAM boost-clock history early
        warm = consts.tile([128, 2, 512], dt.float8e4)
        nc.gpsimd.memset(warm[:], 0.0)
        nc.sync.dma_start(out=w13_sb[:, 0], in_=w13[:, 0])
        nc.sync.dma_start(out=xt_sb[:, 0], in_=xt[0])
        nc.sync.dma_start(out=b13_sb[:], in_=b13[:])
        nc.sync.dma_start(out=w13_sb[:, 1], in_=w13[:, 1])
        nc.sync.dma_start(out=w2_sb[:], in_=w2[:])
        nc.vector.memset(ones_sb[:], 1.0)
        nc.sync.dma_start(out=xt_sb[:, 1], in_=xt[1])
        xt_v = xt.rearrange("g p c n -> p g c n")
        if FP8_WSUM:
            xa_v = xa.rearrange("g p c r d -> p g c r d")
        else:
            xa_v = xa.rearrange("g p c d -> p g c d")
        nc.sync.dma_start(out=xa_sb[:, 0:2], in_=xa_v[:, 0:2])
        nc.sync.dma_start(out=xt_sb[:, 2:4], in_=xt_v[:, 2:4])
        nc.sync.dma_start(out=xa_sb[:, 2:4], in_=xa_v[:, 2:4])
        nc.sync.dma_start(out=xt_sb[:, 4:6], in_=xt_v[:, 4:6])
        nc.sync.dma_start(out=xa_sb[:, 4:6], in_=xa_v[:, 4:6])
        nc.sync.dma_start(out=xt_sb[:, 6:8], in_=xt_v[:, 6:8])
        nc.sync.dma_start(out=xa_sb[:, 6:8], in_=xa_v[:, 6:8])

        # a_ps: one bank, two bag slots rotating; columns 64+ of each
        # slot hold the softmax-denominator matmul outputs (same bank,
        # written only after the slot's A accumulation groups close)
        a_ps = psA.tile([128, 2, 68], dt.float32)
        # output staging: per-bag copies land here; two DMAs at the end
        u_stage = consts.tile([K, BPC, D], dt.float32)
        z_stage = consts.tile([128, BPC, 2] if FP8_WSUM else [16 * K, BPC], dt.float32)

        warm_ps = psU.tile([K, D], dt.float32, tag="u")
        for _ in range(6):
            nc.tensor.matmul(warm_ps[:], warm[:, :, 0:K], warm[:],
                             start=True, stop=True, perf_mode=DR)

        state = {}  # bag -> (u_ps, e_bag)

        def wsum_group(wg):
            """Weighted-sum matmuls for n-tile wg (skew 2)."""
            wstate = state.get(wg // NTB) if wg is not None and wg >= 0 else None
            if wstate is None:
                return
            wu, we = wstate[0], wstate[1]
            wh = wg % NTB
            if FP8_WSUM:
                for jp in range(CPT // 2):
                    cc = wh * (CPT // 2) + jp
                    nc.tensor.matmul(
                        wu[:],
                        we[:, wh * CPT + 2 * jp: wh * CPT + 2 * jp + 2, 0:K],
                        xa_sb[:, wg, jp],
                        start=(cc == 0), stop=(cc == CPT - 1),
                        perf_mode=DR,
                    )
            else:
                for j in range(CPT):
                    cc = wh * CPT + j
                    nc.tensor.matmul(
                        wu[:],
                        we[:, wh * CPT + j, :],
                        xa_sb[:, wg, j],
                        start=(cc == 0), stop=(cc == 2 * CPT - 1),
                    )

        def logits(nt):
            """16 fp8 DoubleRow matmuls + 4 two-bank ACTs + 2 gated pairs
            for n-tile nt. t = tanh(h1+b1); s = sigmoid = 0.5(1+tanh(.5 h3
            + .5 b3)); gated = t*s+t with the 0.5 folded into W2. The
            skewed weighted-sum group slots in after the second h-chunk,
            covering the PE wait for the first ACT to release its PSUM."""
            ts = {}
            for i, (hc, branch) in enumerate(((0, 0), (2, 1), (1, 0), (3, 1))):
                if i == 2:
                    wsum_group(nt - 1)
                    if (nt - 1) % NTB == NTB - 1 and nt >= 2:
                        bag_finish((nt - 1) // NTB)
                h_ps = psH.tile([128, 2, 512], dt.float32, tag="h")
                for half in range(2):
                    for dc2 in range(2):
                        nc.tensor.matmul(
                            h_ps[:, half],
                            w13_sb[:, dc2, :, hc * 128:(hc + 1) * 128],
                            xt_sb[:, nt, 2 * dc2:2 * dc2 + 2, half * 512:(half + 1) * 512],
                            start=(dc2 == 0), stop=(dc2 == 1),
                            perf_mode=DR,
                        )
                t = tsp.tile([128, 2, 512], dt.bfloat16, tag="ts")
                nc.scalar.activation(
                    t[:], h_ps[:], AF.Tanh, bias=b13_sb[:, hc:hc + 1],
                    scale=(1.0 / 16.0 if branch == 0 else 0.5 / 16.0),
                )
                ts[hc] = t
                if branch == 1:  # both branches of this pair done
                    # gated in 512-col halves: subtile deps let the first
                    # A-matmul subs start one vector-op earlier
                    pair = hc - 2
                    apt = app.tile([128, NTILE], dt.bfloat16, tag="ap")
                    tt = ts[pair].rearrange("p a b -> p (a b)")
                    ss = ts[hc].rearrange("p a b -> p (a b)")
                    for hhf in range(2):
                        sl = slice(hhf * 512, (hhf + 1) * 512)
                        nc.vector.tensor_mul(out=apt[:, sl], in0=tt[:, sl], in1=ss[:, sl])
                        nc.vector.tensor_add(out=apt[:, sl], in0=apt[:, sl], in1=tt[:, sl])
                    ts[pair + 10] = apt

            return ts[10], ts[11]

        def a_block(nt, ap0, ap1):
            """A-matmuls of n-tile nt (skew 1), exp, and the per-bag
            softmax denominator."""
            bag, half = nt // NTB, nt % NTB
            slot = bag % 2
            if half == 0:
                e_bag = epool.tile(
                    [128, 16, 16 if FP8_WSUM else K],
                    dt.float8e4 if FP8_WSUM else dt.bfloat16, tag="e")
                u_ps = psU.tile([K, D], dt.float32, tag="u")
                state[bag] = (u_ps, e_bag)

            aps = (ap0, ap1)
            for s in range(CPT):
                c = half * CPT + s
                for pair in range(2):
                    nc.tensor.matmul(
                        a_ps[:, slot, K * c:K * (c + 1)],
                        aps[pair][:, s * 128:(s + 1) * 128],
                        w2_sb[:, pair, :],
                        start=(pair == 0), stop=(pair == 1),
                    )

            # exp(logits) for this n-tile -> e_bag columns (fp8 when the
            # weighted sum runs in DoubleRow; weights and Z stay consistent)
            _, e_bag = state[bag]
            nc.scalar.activation(
                e_bag[:, half * CPT:(half + 1) * CPT, 0:K],
                a_ps[:, slot, 32 * half:32 * (half + 1)].rearrange("p (c k) -> p c k", k=K),
                AF.Exp,
            )
            if half == 1:
                # softmax denominator: ones-matmuls over the bag's exps
                # (e_bag stationary, ones moving -> per-(chunk, head)
                # partial sums on partitions; host adds the partials).
                # The stationary must be a single contiguous free dim, so
                # fp8 mode sums two [128, 128] halves (padding columns
                # produce junk rows the host ignores).
                if FP8_WSUM:
                    z_ps = a_ps[:, slot, 64:66]
                    eb2 = e_bag.rearrange("p c k -> p (c k)")
                    for hh in range(2):
                        nc.tensor.matmul(z_ps[:, hh:hh + 1], eb2[:, hh * 128:(hh + 1) * 128],
                                         ones_sb[:], start=True, stop=True)
                    nc.vector.tensor_scalar_add(out=z_stage[:, bag], in0=z_ps, scalar1=0.0)
                else:
                    z_ps = a_ps[0:64, slot, 64:65]
                    nc.tensor.matmul(z_ps, e_bag[:, :, 0:K], ones_sb[:], start=True, stop=True)
                    nc.vector.tensor_scalar_add(out=z_stage[:, bag:bag + 1], in0=z_ps, scalar1=0.0)

        def bag_finish(bag):
            u_ps = state[bag][0]
            nc.vector.tensor_scalar_add(out=u_stage[:, bag], in0=u_ps[:], scalar1=0.0)
            nc.sync.dma_start(out=uout[:, bag], in_=u_stage[:, bag])

        # main pipeline: iteration it runs A/exp/z of n-tile it-1, then
        # logits of it (with the weighted sums of it-1 nested mid-tile)
        prev_ap = None
        for it in range(NT):
            if it >= 1:
                a_block(it - 1, prev_ap[0], prev_ap[1])
            prev_ap = logits(it)
        a_block(NT - 1, prev_ap[0], prev_ap[1])
        wsum_group(NT - 1)
        bag_finish((NT - 1) // NTB)

        nc.sync.dma_start(out=zout[:], in_=z_stage[:])

    nc.compile()
    return nc


def get_nc():
    if "nc" not in _CACHE:
        _CACHE["nc"] = _build_nc()
    return _CACHE["nc"]


def make_in_maps(x, W1, b1, W3, b3, W2, b2):
    x = np.asarray(x, dtype=np.float32)
    W1 = np.asarray(W1, dtype=np.float32)
    W3 = np.asarray(W3, dtype=np.float32)
    W2 = np.asarray(W2, dtype=np.float32)
    b1 = np.asarray(b1, dtype=np.float32)
    b3 = np.asarray(b3, dtype=np.float32)

    # [W1 | W3] packed for fp8 DoubleRow: [p, dc2, r, h'] = 16*W13[dc2*256+r*128+p, h']
    w13 = np.concatenate([W1, W3], axis=1)          # [512, 512]
    w13_t = np.ascontiguousarray(
        (16.0 * w13).reshape(2, 2, 128, 2 * H).transpose(2, 0, 1, 3)
    ).astype(_FP8)
    # 0.5 * W2 with layout [p, hc, k]
    w2_t = np.ascontiguousarray(
        (0.5 * W2).reshape(2, 128, K).transpose(1, 0, 2)
    ).astype(_BF16)
    # biases [p, j]: j in {0,1} -> b1 chunks, {2,3} -> 0.5*b3 chunks
    b13 = np.concatenate([b1, 0.5 * b3]).reshape(4, 128).T
    b13 = np.ascontiguousarray(b13, dtype=np.float32)

    in_maps = []
    for c in range(NCORES):
        xc = x[c * R:(c + 1) * R]                   # [8192, 512] fp32
        # [nt, p, 2*dc2+r, n] = xc[nt*NTILE+n, dc2*256+r*128+p]
        xt_np = np.ascontiguousarray(
            xc.T.reshape(2, 2, 128, NT, NTILE).transpose(3, 2, 0, 1, 4).reshape(NT, 128, 4, NTILE)
        ).astype(_FP8)
        if FP8_WSUM:
            xa_np = np.ascontiguousarray(
                xc.reshape(NT, CPT // 2, 2, 128, D).transpose(0, 3, 1, 2, 4)
            ).astype(_FP8)
        else:
            xa_np = np.ascontiguousarray(
                xc.reshape(NT, CPT, 128, D).transpose(0, 2, 1, 3)
            ).astype(_BF16)
        in_maps.append(
            {"xt": xt_np, "xa": xa_np, "w13": w13_t, "w2": w2_t, "b13": b13}
        )
    return in_maps


def postprocess(res):
    """Assemble [B, K*D] output from per-core unnormalized sums."""
    out = np.empty((B, K * D), dtype=np.float32)
    for c in range(NCORES):
        u = res.results[c]["uout"].transpose(1, 0, 2)  # [BPC, K, D]
        z2 = res.results[c]["zout"]
        if FP8_WSUM:                                   # [128, BPC, 2]
            z = z2.reshape(8, 16, BPC, 2)[:, 0:K].sum(axis=(0, 3)).T
        else:                                          # [64, BPC]
            z = z2.reshape(16, K, BPC).sum(axis=0).T   # [BPC, K]
        out[c * BPC:(c + 1) * BPC] = (u / z[:, :, None]).reshape(BPC, K * D)
    return out


def kernel(x, W1, b1, W3, b3, W2, b2, bag_lengths):
    from concourse.bass_utils import run_bass_kernel_spmd

    nc = get_nc()
    in_maps = make_in_maps(x, W1, b1, W3, b3, W2, b2)
    res = run_bass_kernel_spmd(nc, in_maps, list(range(NCORES)))
    return postprocess(res)


# revision 39
# speedup vs baseline: 1.2135x; 1.2135x over previous
"""MIL gated-attention pooling kernel for Trainium2 (8 NeuronCores, SPMD).

Problem (per reference):
    A_pre = tanh(x@W1 + b1) * sigmoid(x@W3 + b3)      # [N, H]
    A     = A_pre @ W2 + b2                           # [N, K]
    P     = softmax over instances per (bag, head)    # [B, K, L]
    out   = einsum('bkl,bld->bkd', P, x) -> [B, K*D]

Shapes hardcoded: B=32 bags, L=2048 instances/bag, D=512, H=256, K=4.
Sharding: data-parallel over bags, 4 bags (8192 rows) per core, weights
replicated. No cross-core communication.

Device design (v2):
  - All input DMAs are issued upfront from the sync queue in consumption
    order; HBM tensors are pre-tiled on the host so each transfer is a
    few KB contiguous per partition (the DGE fans one dma_start across
    all 16 hardware queues).
  - Logits path: fp8 DoubleRow matmuls against a host-pretransposed,
    pre-packed xT; sigmoid(h) = 0.5*(1+tanh(h/2)) with the 0.5 folded
    into W2; b2 and softmax max-subtraction dropped (cancel in softmax).
  - NTILE=1024 instances per pipeline step; each activation covers a
    [128, 2, 512] PSUM span (two banks) to amortize the ~350ns fixed
    ACT cost.
  - Softmax denominator comes from one ones-vector matmul per bag over
    the exp tile; normalization (u/z) happens on the host, so the
    device tail is just two small copies + DMAs per bag.
  - Two-level software pipeline: the A=ap@W2 matmuls of n-tile i run
    during iteration i+1, the exp-weighted sums of n-tile i during
    iteration i+2, interleaved so PE weight loads hide under streams.
  - FP8_WSUM: weighted sum also in fp8 DoubleRow (x and exp(A) both
    fp8) — halves both its PE streaming time and the xa HBM traffic.
"""

import numpy as np
import ml_dtypes
from contextlib import ExitStack

B, L, D, H, K = 32, 2048, 512, 256, 4
NCORES = 8
BPC = B // NCORES        # bags per core = 4
R = BPC * L              # rows per core = 8192
NTILE = 1024             # instances per pipeline step
NT = R // NTILE          # n-tiles per core = 8
NTB = 2                  # n-tiles per bag
CPT = NTILE // 128       # 128-row chunks per n-tile = 8

_BF16 = ml_dtypes.bfloat16
_FP8 = ml_dtypes.float8_e4m3
FP8_WSUM = True          # fp8 DoubleRow for the weighted sum too
_CACHE = {}


def _build_nc():
    import concourse.bacc as bacc
    import concourse.tile as tile
    import concourse.mybir as mybir
    import concourse.bass as bass

    dt = mybir.dt
    AF = mybir.ActivationFunctionType
    DR = mybir.MatmulPerfMode.DoubleRow

    nc = bacc.Bacc("TRN2", target_bir_lowering=False, debug=False)
    # [nt, p, 2*dc2+r, n] = xT[dc2*256 + r*128 + p, nt*NTILE + n] (fp8, x*1)
    xt = nc.dram_tensor("xt", [NT, 128, 4, NTILE], dt.float8e4, kind="ExternalInput").ap()
    if FP8_WSUM:
        # [g, p, jp, r, d] = x[g*NTILE + jp*256 + r*128 + p, d] (fp8)
        xa = nc.dram_tensor("xa", [NT, 128, CPT // 2, 2, D], dt.float8e4, kind="ExternalInput").ap()
    else:
        # [g, p, j, d] = x[g*NTILE + j*128 + p, d] (bf16)
        xa = nc.dram_tensor("xa", [NT, 128, CPT, D], dt.bfloat16, kind="ExternalInput").ap()
    # [p, dc2, r, h'] = 16*W13[dc2*256 + r*128 + p, h']
    w13 = nc.dram_tensor("w13", [128, 2, 2, 2 * H], dt.float8e4, kind="ExternalInput").ap()
    w2 = nc.dram_tensor("w2", [128, 2, K], dt.bfloat16, kind="ExternalInput").ap()
    b13 = nc.dram_tensor("b13", [128, 4], dt.float32, kind="ExternalInput").ap()
    uout = nc.dram_tensor("uout", [K, BPC, D], dt.float32, kind="ExternalOutput").ap()
    zshape = [128, BPC, 2] if FP8_WSUM else [16 * K, BPC]
    zout = nc.dram_tensor("zout", zshape, dt.float32, kind="ExternalOutput").ap()

    with tile.TileContext(nc) as tc, ExitStack() as ctx:
        consts = ctx.enter_context(tc.tile_pool(name="consts", bufs=1))
        tsp = ctx.enter_context(tc.tile_pool(name="tsp", bufs=4))
        app = ctx.enter_context(tc.tile_pool(name="app", bufs=4))
        epool = ctx.enter_context(tc.tile_pool(name="epool", bufs=2))
        opool = ctx.enter_context(tc.tile_pool(name="opool", bufs=2))

        psH = ctx.enter_context(tc.tile_pool(name="psH", bufs=3, space=bass.MemorySpace.PSUM))
        psA = ctx.enter_context(tc.tile_pool(name="psA", bufs=1, space=bass.MemorySpace.PSUM))
        psU = ctx.enter_context(tc.tile_pool(name="psU", bufs=1, space=bass.MemorySpace.PSUM))

        # SBUF-resident inputs: everything fits, so issue every input DMA
        # upfront in consumption order (each one fans out over the DGE
        # queues; completion order tracks trigger order).
        w13_sb = consts.tile([128, 2, 2, 2 * H], dt.float8e4)
        w2_sb = consts.tile([128, 2, K], dt.bfloat16)
        b13_sb = consts.tile([128, 4], dt.float32)
        xt_sb = consts.tile([128, NT, 4, NTILE], dt.float8e4)
        if FP8_WSUM:
            xa_sb = consts.tile([128, NT, CPT // 2, 2, D], dt.float8e4)
        else:
            xa_sb = consts.tile([128, NT, CPT, D], dt.bfloat16)
        ones_sb = consts.tile([128, 1], dt.float8e4 if FP8_WSUM else dt.bfloat16)

        # the dummy tanh is scalar's first instruction: it pulls the
        # ~1.3us activation-table load into the input-DMA wait window
        # instead of letting it block the first real activation.
        scratch = consts.tile([128, 1], dt.float32)
        nc.scalar.activation(scratch[:], scratch[:], AF.Tanh)
        # PE warmup: dummy matmuls on zeroed scratch fill the first-DMA
        # wait window and start the HAM boost-clock history early
        warm = consts.tile([128, 2, 512], dt.float8e4)
        nc.gpsimd.memset(warm[:], 0.0)
        nc.sync.dma_start(out=w13_sb[:, 0], in_=w13[:, 0])
        nc.sync.dma_start(out=xt_sb[:, 0], in_=xt[0])
        nc.sync.dma_start(out=b13_sb[:], in_=b13[:])
        nc.sync.dma_start(out=w13_sb[:, 1], in_=w13[:, 1])
        nc.sync.dma_start(out=w2_sb[:], in_=w2[:])
        nc.vector.memset(ones_sb[:], 1.0)
        nc.sync.dma_start(out=xt_sb[:, 1], in_=xt[1])
        xt_v = xt.rearrange("g p c n -> p g c n")
        if FP8_WSUM:
            xa_v = xa.rearrange("g p c r d -> p g c r d")
        else:
            xa_v = xa.rearrange("g p c d -> p g c d")
        nc.sync.dma_start(out=xa_sb[:, 0:2], in_=xa_v[:, 0:2])
        nc.sync.dma_start(out=xt_sb[:, 2:4], in_=xt_v[:, 2:4])
        nc.sync.dma_start(out=xa_sb[:, 2:4], in_=xa_v[:, 2:4])
        nc.sync.dma_start(out=xt_sb[:, 4:6], in_=xt_v[:, 4:6])
        nc.sync.dma_start(out=xa_sb[:, 4:6], in_=xa_v[:, 4:6])
        nc.sync.dma_start(out=xt_sb[:, 6:8], in_=xt_v[:, 6:8])
        nc.sync.dma_start(out=xa_sb[:, 6:8], in_=xa_v[:, 6:8])

        # a_ps: one bank, two bag slots rotating; columns 64+ of each
        # slot hold the softmax-denominator matmul outputs (same bank,
        # written only after the slot's A accumulation groups close)
        a_ps = psA.tile([128, 2, 68], dt.float32)
        # output staging: per-bag copies land here; two DMAs at the end
        u_stage = consts.tile([K, BPC, D], dt.float32)
        z_stage = consts.tile([128, BPC, 2] if FP8_WSUM else [16 * K, BPC], dt.float32)

        warm_ps = psU.tile([K, D], dt.float32, tag="u")
        for _ in range(12):
            nc.tensor.matmul(warm_ps[:], warm[:, :, 0:K], warm[:],
                             start=True, stop=True, perf_mode=DR)

        state = {}  # bag -> (u_ps, e_bag)

        def wsum_group(wg):
            """Weighted-sum matmuls for n-tile wg (skew 2)."""
            wstate = state.get(wg // NTB) if wg is not None and wg >= 0 else None
            if wstate is None:
                return
            wu, we = wstate[0], wstate[1]
            wh = wg % NTB
            if FP8_WSUM:
                for jp in range(CPT // 2):
                    cc = wh * (CPT // 2) + jp
                    nc.tensor.matmul(
                        wu[:],
                        we[:, wh * CPT + 2 * jp: wh * CPT + 2 * jp + 2, 0:K],
                        xa_sb[:, wg, jp],
                        start=(cc == 0), stop=(cc == CPT - 1),
                        perf_mode=DR,
                    )
            else:
                for j in range(CPT):
                    cc = wh * CPT + j
                    nc.tensor.matmul(
                        wu[:],
                        we[:, wh * CPT + j, :],
                        xa_sb[:, wg, j],
                        start=(cc == 0), stop=(cc == 2 * CPT - 1),
                    )

        def logits(nt):
            """16 fp8 DoubleRow matmuls + 4 two-bank ACTs + 2 gated pairs
            for n-tile nt. t = tanh(h1+b1); s = sigmoid = 0.5(1+tanh(.5 h3
            + .5 b3)); gated = t*s+t with the 0.5 folded into W2. The
            skewed weighted-sum group slots in after the second h-chunk,
            covering the PE wait for the first ACT to release its PSUM."""
            ts = {}
            for i, (hc, branch) in enumerate(((0, 0), (2, 1), (1, 0), (3, 1))):
                if i == 2:
                    wsum_group(nt - 1)
                    if (nt - 1) % NTB == NTB - 1 and nt >= 2:
                        bag_finish((nt - 1) // NTB)
                h_ps = psH.tile([128, 2, 512], dt.float32, tag="h")
                for half in range(2):
                    for dc2 in range(2):
                        nc.tensor.matmul(
                            h_ps[:, half],
                            w13_sb[:, dc2, :, hc * 128:(hc + 1) * 128],
                            xt_sb[:, nt, 2 * dc2:2 * dc2 + 2, half * 512:(half + 1) * 512],
                            start=(dc2 == 0), stop=(dc2 == 1),
                            perf_mode=DR,
                        )
                t = tsp.tile([128, 2, 512], dt.bfloat16, tag="ts")
                nc.scalar.activation(
                    t[:], h_ps[:], AF.Tanh, bias=b13_sb[:, hc:hc + 1],
                    scale=(1.0 / 16.0 if branch == 0 else 0.5 / 16.0),
                )
                ts[hc] = t
                if branch == 1:  # both branches of this pair done
                    # gated in 512-col halves: subtile deps let the first
                    # A-matmul subs start one vector-op earlier
                    pair = hc - 2
                    apt = app.tile([128, NTILE], dt.bfloat16, tag="ap")
                    tt = ts[pair].rearrange("p a b -> p (a b)")
                    ss = ts[hc].rearrange("p a b -> p (a b)")
                    for hhf in range(2):
                        sl = slice(hhf * 512, (hhf + 1) * 512)
                        nc.vector.tensor_mul(out=apt[:, sl], in0=tt[:, sl], in1=ss[:, sl])
                        nc.vector.tensor_add(out=apt[:, sl], in0=apt[:, sl], in1=tt[:, sl])
                    ts[pair + 10] = apt

            return ts[10], ts[11]

        def a_block(nt, ap0, ap1):
            """A-matmuls of n-tile nt (skew 1), exp, and the per-bag
            softmax denominator."""
            bag, half = nt // NTB, nt % NTB
            slot = bag % 2
            if half == 0:
                e_bag = epool.tile(
                    [128, 16, 16 if FP8_WSUM else K],
                    dt.float8e4 if FP8_WSUM else dt.bfloat16, tag="e")
                u_ps = psU.tile([K, D], dt.float32, tag="u")
                state[bag] = (u_ps, e_bag)

            aps = (ap0, ap1)
            for s in range(CPT):
                c = half * CPT + s
                for pair in range(2):
                    nc.tensor.matmul(
                        a_ps[:, slot, K * c:K * (c + 1)],
                        aps[pair][:, s * 128:(s + 1) * 128],
                        w2_sb[:, pair, :],
                        start=(pair == 0), stop=(pair == 1),
                    )

            # exp(logits) for this n-tile -> e_bag columns (fp8 when the
            # weighted sum runs in DoubleRow; weights and Z stay consistent)
            _, e_bag = state[bag]
            nc.scalar.activation(
                e_bag[:, half * CPT:(half + 1) * CPT, 0:K],
                a_ps[:, slot, 32 * half:32 * (half + 1)].rearrange("p (c k) -> p c k", k=K),
                AF.Exp,
            )
            if half == 1:
                # softmax denominator: ones-matmuls over the bag's exps
                # (e_bag stationary, ones moving -> per-(chunk, head)
                # partial sums on partitions; host adds the partials).
                # The stationary must be a single contiguous free dim, so
                # fp8 mode sums two [128, 128] halves (padding columns
                # produce junk rows the host ignores).
                if FP8_WSUM:
                    z_ps = a_ps[:, slot, 64:66]
                    eb2 = e_bag.rearrange("p c k -> p (c k)")
                    for hh in range(2):
                        nc.tensor.matmul(z_ps[:, hh:hh + 1], eb2[:, hh * 128:(hh + 1) * 128],
                                         ones_sb[:], start=True, stop=True)
                    nc.vector.tensor_scalar_add(out=z_stage[:, bag], in0=z_ps, scalar1=0.0)
                else:
                    z_ps = a_ps[0:64, slot, 64:65]
                    nc.tensor.matmul(z_ps, e_bag[:, :, 0:K], ones_sb[:], start=True, stop=True)
                    nc.vector.tensor_scalar_add(out=z_stage[:, bag:bag + 1], in0=z_ps, scalar1=0.0)

        def bag_finish(bag):
            u_ps = state[bag][0]
            nc.vector.tensor_scalar_add(out=u_stage[:, bag], in0=u_ps[:], scalar1=0.0)
            nc.sync.dma_start(out=uout[:, bag], in_=u_stage[:, bag])

        # main pipeline: iteration it runs A/exp/z of n-tile it-1, then
        # logits of it (with the weighted sums of it-1 nested mid-tile)
        prev_ap = None
        for it in range(NT):
            if it >= 1:
                a_block(it - 1, prev_ap[0], prev_ap[1])
            prev_ap = logits(it)
        a_block(NT - 1, prev_ap[0], prev_ap[1])
        wsum_group(NT - 1)
        bag_finish((NT - 1) // NTB)

        nc.sync.dma_start(out=zout[:], in_=z_stage[:])

    nc.compile()
    return nc


def get_nc():
    if "nc" not in _CACHE:
        _CACHE["nc"] = _build_nc()
    return _CACHE["nc"]


def make_in_maps(x, W1, b1, W3, b3, W2, b2):
    x = np.asarray(x, dtype=np.float32)
    W1 = np.asarray(W1, dtype=np.float32)
    W3 = np.asarray(W3, dtype=np.float32)
    W2 = np.asarray(W2, dtype=np.float32)
    b1 = np.asarray(b1, dtype=np.float32)
    b3 = np.asarray(b3, dtype=np.float32)

    # [W1 | W3] packed for fp8 DoubleRow: [p, dc2, r, h'] = 16*W13[dc2*256+r*128+p, h']
    w13 = np.concatenate([W1, W3], axis=1)          # [512, 512]
    w13_t = np.ascontiguousarray(
        (16.0 * w13).reshape(2, 2, 128, 2 * H).transpose(2, 0, 1, 3)
    ).astype(_FP8)
    # 0.5 * W2 with layout [p, hc, k]
    w2_t = np.ascontiguousarray(
        (0.5 * W2).reshape(2, 128, K).transpose(1, 0, 2)
    ).astype(_BF16)
    # biases [p, j]: j in {0,1} -> b1 chunks, {2,3} -> 0.5*b3 chunks
    b13 = np.concatenate([b1, 0.5 * b3]).reshape(4, 128).T
    b13 = np.ascontiguousarray(b13, dtype=np.float32)

    in_maps = []
    for c in range(NCORES):
        xc = x[c * R:(c + 1) * R]                   # [8192, 512] fp32
        # [nt, p, 2*dc2+r, n] = xc[nt*NTILE+n, dc2*256+r*128+p]
        xt_np = np.ascontiguousarray(
            xc.T.reshape(2, 2, 128, NT, NTILE).transpose(3, 2, 0, 1, 4).reshape(NT, 128, 4, NTILE)
        ).astype(_FP8)
        if FP8_WSUM:
            xa_np = np.ascontiguousarray(
                xc.reshape(NT, CPT // 2, 2, 128, D).transpose(0, 3, 1, 2, 4)
            ).astype(_FP8)
        else:
            xa_np = np.ascontiguousarray(
                xc.reshape(NT, CPT, 128, D).transpose(0, 2, 1, 3)
            ).astype(_BF16)
        in_maps.append(
            {"xt": xt_np, "xa": xa_np, "w13": w13_t, "w2": w2_t, "b13": b13}
        )
    return in_maps


def postprocess(res):
    """Assemble [B, K*D] output from per-core unnormalized sums."""
    out = np.empty((B, K * D), dtype=np.float32)
    for c in range(NCORES):
        u = res.results[c]["uout"].transpose(1, 0, 2)  # [BPC, K, D]
        z2 = res.results[c]["zout"]
        if FP8_WSUM:                                   # [128, BPC, 2]
            z = z2.reshape(8, 16, BPC, 2)[:, 0:K].sum(axis=(0, 3)).T
        else:                                          # [64, BPC]
            z = z2.reshape(16, K, BPC).sum(axis=0).T   # [BPC, K]
        out[c * BPC:(c + 1) * BPC] = (u / z[:, :, None]).reshape(BPC, K * D)
    return out


def kernel(x, W1, b1, W3, b3, W2, b2, bag_lengths):
    from concourse.bass_utils import run_bass_kernel_spmd

    nc = get_nc()
    in_maps = make_in_maps(x, W1, b1, W3, b3, W2, b2)
    res = run_bass_kernel_spmd(nc, in_maps, list(range(NCORES)))
    return postprocess(res)
